# revision 17
# baseline (speedup 1.0000x reference)
"""Multi-head attention unit (proj + softmax attention + out-proj + bias + GELU)
for Trainium2, SPMD across 8 NeuronCores.

Sharding: core c = (batch b=c//2, query-half j=c%2). Each core computes all 16
heads for its 1024 query rows of batch b; k/v projections for the full 2048
keys of batch b are computed per-core (duplicated across the 2 cores sharing a
batch - cheaper than cross-core collectives).

Layout strategy: all activations/weights are transposed on the host so every
matmul operand arrives in d-major (contraction-on-partition) layout:
  - scores are computed TRANSPOSED [kpos, qpos] so the AV matmul needs no
    on-chip transpose of the softmax matrix;
  - v is stored in natural [kpos, d] layout with a ones-column appended, so
    the AV matmul's 65th output row is the softmax denominator for free;
  - ctx comes out d-major [d, qpos], which is exactly the stationary layout
    the output projection wants; bias is added with a K=1 ones-row matmul and
    normalization is a reciprocal + K=1 outer-product broadcast + DVE mult.
Matmul compute dtype: bf16 by default (PSUM accumulation is fp32), COMPUTE_DT
env var can select f32r (TF32-like) for higher precision at ~2.5x PE cost.
mask is all-ones by construction (spec fill=ones) -> ignored.
"""

import os

import numpy as np

B, S, D, NH = 4, 2048, 1024, 16
HD = D // NH          # 64
NCORES = 8
QLEN = S // 2         # 1024 query rows per core
NQB = QLEN // 512     # q blocks of 512
NKT = S // 128        # 16 kpos tiles
NDC = D // 128        # 8 contraction chunks
COMPUTE_DT = os.environ.get("COMPUTE_DT", "bf16")
USE_ALLGATHER = os.environ.get("USE_ALLGATHER", "1") == "1"
USE_PBCAST = os.environ.get("USE_PBCAST", "1") == "1"

_CACHED_NC = None


def _build():
    import concourse.bacc as bacc
    import concourse.mybir as mybir
    import concourse.tile as tile

    F32 = mybir.dt.float32
    CDT = mybir.dt.bfloat16 if COMPUTE_DT == "bf16" else mybir.dt.float32r
    ACT = mybir.ActivationFunctionType

    nc = bacc.Bacc("TRN2", target_bir_lowering=False, debug=False)

    qT_in = nc.dram_tensor("qT_in", [D, QLEN], CDT, kind="ExternalInput")
    kT_in = nc.dram_tensor("kT_in", [D, QLEN], CDT, kind="ExternalInput")
    vT_in = nc.dram_tensor("vT_in", [D, QLEN], CDT, kind="ExternalInput")
    WqT = nc.dram_tensor("WqT", [D, D], CDT, kind="ExternalInput")
    WkT = nc.dram_tensor("WkT", [D, D], CDT, kind="ExternalInput")
    WvT = nc.dram_tensor("WvT", [D, D], CDT, kind="ExternalInput")
    WoT = nc.dram_tensor("WoT", [D, D], CDT, kind="ExternalInput")
    b_o = nc.dram_tensor("b_o", [1, D], CDT, kind="ExternalInput")
    out = nc.dram_tensor("out", [QLEN, D], F32, kind="ExternalOutput")
    KHALF = 128 * NDC * QLEN           # kT half elems
    VHALF = 128 * (NKT // 2) * NH * (HD + 1)
    kT_b = nc.dram_tensor("kT_b", [KHALF], CDT)
    kT_g = nc.dram_tensor("kT_g", [2, KHALF], CDT)
    va_b = nc.dram_tensor("va_b", [VHALF], CDT)
    va_g = nc.dram_tensor("va_g", [2, VHALF], CDT)
    PAIR_GROUPS = [[0, 1], [2, 3], [4, 5], [6, 7]]

    from contextlib import ExitStack
    with tile.TileContext(nc) as tc, ExitStack() as es:
        ep = es.enter_context
        cpool = ep(tc.tile_pool(name="consts", bufs=1))
        wpool = ep(tc.tile_pool(name="wt", bufs=2))
        xpool = ep(tc.tile_pool(name="xin", bufs=3))
        vinpool = ep(tc.tile_pool(name="vin", bufs=3))
        respool = ep(tc.tile_pool(name="res", bufs=1))
        epool = ep(tc.tile_pool(name="exp", bufs=6))
        npool = ep(tc.tile_pool(name="norm", bufs=2))
        ctxpool_sb = ep(tc.tile_pool(name="ctxn", bufs=1))
        opool = ep(tc.tile_pool(name="osb", bufs=3))
        mmps = ep(tc.tile_pool(name="mm_ps", bufs=2, space="PSUM"))
        sps = ep(tc.tile_pool(name="s_ps", bufs=2, space="PSUM"))
        cps = ep(tc.tile_pool(name="ctx_ps", bufs=2, space="PSUM"))
        if True:
            # ---- constants ----
            ones_f = cpool.tile([128, 128], F32, tag="ones_f")
            nc.gpsimd.memset(ones_f[:], 1.0)
            ones = cpool.tile([128, 128], CDT, tag="ones_r")
            nc.vector.tensor_copy(ones[:], ones_f[:])

            # ---- DRAM intermediates ----
            # SBUF-resident projection outputs (d = pair*128 + p for q/k)
            qT_sb = respool.tile([128, NDC, QLEN], CDT, tag="qT_sb", name="qT_sb")
            kT_sb = respool.tile([128, NDC, S], CDT, tag="kT_sb", name="kT_sb")
            # va_sb[p, kt, h, c]: c 0..63 = v_nat[kt*128+p, h*64+c], c=64 -> 1.0
            va_sb = respool.tile([128, NKT, NH, HD + 1], CDT, tag="va_sb",
                                 name="va_sb")

            # ======== stage 1a: q/k projections (out = d-major) ========
            # loop order: stationary (W chunk) outer, 2 moving blocks inner so
            # each LDWEIGHTS serves 2 matmuls.
            with nc.named_scope("proj_qk"):
                for name, w_dram, x_dram, x_sbres, ncols in (
                    ("k", WkT, kT_in, kT_sb, QLEN),
                    ("q", WqT, qT_in, qT_sb, QLEN),
                ):
                    w_sb = wpool.tile([128, NDC, D], CDT, tag="wt")
                    nc.sync.dma_start(
                        w_sb[:], w_dram[:].rearrange("(dc p) d -> p dc d", p=128)
                    )
                    for xb in range(ncols // 512):
                        x_sb = xpool.tile([128, NDC, 512], CDT, tag="xin")
                        nc.sync.dma_start(
                            x_sb[:],
                            x_dram[:, xb * 512:(xb + 1) * 512].rearrange(
                                "(dc p) s -> p dc s", p=128
                            ),
                        )
                        for dt_ in range(NDC):
                            ps = mmps.tile([128, 512], F32, name="pp", tag="mmp")
                            for dc in range(NDC):
                                nc.tensor.matmul(
                                    ps[:],
                                    w_sb[:, dc, dt_ * 128:(dt_ + 1) * 128],
                                    x_sb[:, dc, :],
                                    start=(dc == 0),
                                    stop=(dc == NDC - 1),
                                )
                            nc.vector.tensor_copy(
                                x_sbres[:, dt_, xb * 512:(xb + 1) * 512], ps[:]
                            )
                    if name == "k" and USE_ALLGATHER:
                        # exchange halves with pair neighbor
                        nc.sync.dma_start(
                            kT_b[:].rearrange("(p dc s) -> p dc s", p=128, dc=NDC),
                            kT_sb[:, :, 0:QLEN],
                        )
                        nc.gpsimd.collective_compute(
                            "AllGather", mybir.AluOpType.bypass,
                            replica_groups=PAIR_GROUPS,
                            ins=[kT_b[:]], outs=[kT_g[:]],
                        )
                        for r in range(2):
                            nc.sync.dma_start(
                                kT_sb[:, :, r * QLEN:(r + 1) * QLEN],
                                kT_g[r].rearrange("(p dc s) -> p dc s",
                                                  p=128, dc=NDC),
                            )

            # ======== stage 1b: v projection (out = natural [kpos, d] + ones) ==
            with nc.named_scope("proj_v"):
                wv_sb = wpool.tile([128, NDC, D], CDT, tag="wt")
                nc.sync.dma_start(
                    wv_sb[:], WvT[:].rearrange("(dc p) d -> p dc d", p=128)
                )
                for kt in range(NKT // 2):
                    vin = vinpool.tile([128, NDC, 128], CDT, tag="vin")
                    nc.sync.dma_start(
                        vin[:],
                        vT_in[:, kt * 128:(kt + 1) * 128].rearrange(
                            "(dc p) s -> p dc s", p=128
                        ),
                    )
                    for dbl in range(2):
                        ps = mmps.tile([128, 512], F32, name="pp", tag="mmp")
                        for dc in range(NDC):
                            nc.tensor.matmul(
                                ps[:],
                                vin[:, dc, :],
                                wv_sb[:, dc, dbl * 512:(dbl + 1) * 512],
                                start=(dc == 0),
                                stop=(dc == NDC - 1),
                            )
                        nc.vector.tensor_copy(
                            va_sb[:, kt, dbl * 8:(dbl + 1) * 8, 0:HD],
                            ps[:].rearrange("p (h c) -> p h c", c=HD),
                        )
                    nc.vector.tensor_copy(va_sb[:, kt, :, HD], ones[:, 0:16])
                nc.sync.dma_start(
                    va_b[:].rearrange("(p kt h c) -> p kt h c", p=128,
                                      kt=NKT // 2, h=NH),
                    va_sb[:, 0:NKT // 2, :, :],
                )
                nc.gpsimd.collective_compute(
                    "AllGather", mybir.AluOpType.bypass,
                    replica_groups=PAIR_GROUPS,
                    ins=[va_b[:]], outs=[va_g[:]],
                )
                for r in range(2):
                    nc.sync.dma_start(
                        va_sb[:, r * (NKT // 2):(r + 1) * (NKT // 2), :, :],
                        va_g[r].rearrange("(p kt h c) -> p kt h c", p=128,
                                          kt=NKT // 2, h=NH),
                    )

            # ======== stage 2: attention per head-pair ========
            ctxn = ctxpool_sb.tile([128, NH // 2, QLEN], CDT, tag="ctxn")
            with nc.named_scope("attn"):
                for pair in range(NH // 2):
                    ktp = kT_sb[:, pair, :]
                    qtp = qT_sb[:, pair, :]
                    vas = [va_sb[:, :, 2 * pair + h_idx, :] for h_idx in range(2)]
                    for h_idx in range(2):
                        base = h_idx * HD
                        for qb in range(NQB):
                            ctx_ps = cps.tile([HD + 1, 512], F32, name="cp", tag="cp")
                            for kt in range(0, NKT, 2):
                                s_ps = sps.tile([128, 1024], F32, name="sp", tag="sp")
                                for k2 in range(2):
                                    nc.tensor.matmul(
                                        s_ps[:, k2 * 512:(k2 + 1) * 512],
                                        ktp[base:base + HD,
                                            (kt + k2) * 128:(kt + k2 + 1) * 128],
                                        qtp[base:base + HD,
                                            qb * 512:(qb + 1) * 512],
                                    )
                                e_sb = epool.tile([128, 1024], CDT, tag="e")
                                nc.scalar.activation(
                                    e_sb[:], s_ps[:], ACT.Exp,
                                    scale=float(HD) ** -0.5,
                                )
                                for k2 in range(2):
                                    nc.tensor.matmul(
                                        ctx_ps[:],
                                        vas[h_idx][:, kt + k2, :],
                                        e_sb[:, k2 * 512:(k2 + 1) * 512],
                                        start=(kt + k2 == 0),
                                        stop=(kt + k2 == NKT - 1),
                                    )
                            # normalize: ctxn[h] = ctx * (1/denom), denom
                            # broadcast over d via K=1 outer-product matmul
                            den_sb = npool.tile([1, 512], F32, tag="den_sb")
                            nc.vector.tensor_copy(den_sb[:], ctx_ps[HD:HD + 1, :])
                            scratch = npool.tile([1, 512], F32, tag="recip_s")
                            nc.vector.reciprocal_approx_fast(
                                out=scratch[:], in_=den_sb[:]
                            )
                            recip = npool.tile([1, 512], CDT, tag="recip")
                            nc.vector.tensor_copy(recip[:], scratch[:])
                            bc_sb = npool.tile([HD, 512], CDT, tag="bc")
                            if USE_PBCAST:
                                nc.gpsimd.partition_broadcast(bc_sb[:], recip[:])
                            else:
                                bc_ps = mmps.tile([HD, 512], F32, name="pp", tag="mmp")
                                nc.tensor.matmul(bc_ps[:], ones[0:1, 0:HD], recip[:])
                                nc.vector.tensor_copy(bc_sb[:], bc_ps[:])
                            nc.vector.tensor_mul(
                                ctxn[base:base + HD, pair,
                                     qb * 512:(qb + 1) * 512],
                                ctx_ps[0:HD, :],
                                bc_sb[:],
                            )

            # ======== stage 3: out-projection + bias + gelu ========
            with nc.named_scope("outproj"):
                wo_sb = wpool.tile([128, NDC, D], CDT, tag="wt")
                nc.sync.dma_start(
                    wo_sb[:], WoT[:].rearrange("(dc p) d -> p dc d", p=128)
                )
                bo_sb = cpool.tile([1, D], CDT, tag="bo")
                nc.sync.dma_start(bo_sb[:], b_o[:])
                for qt in range(QLEN // 128):
                    for dbl in range(2):
                        ps = mmps.tile([128, 512], F32, name="pp", tag="mmp")
                        for pair in range(NH // 2):
                            nc.tensor.matmul(
                                ps[:],
                                ctxn[:, pair, qt * 128:(qt + 1) * 128],
                                wo_sb[:, pair, dbl * 512:(dbl + 1) * 512],
                                start=(pair == 0),
                                stop=False,
                            )
                        nc.tensor.matmul(
                            ps[:],
                            ones[0:1, 0:128],
                            bo_sb[0:1, dbl * 512:(dbl + 1) * 512],
                            start=False,
                            stop=True,
                        )
                        o_sb = opool.tile([128, 512], F32, tag="osb")
                        nc.scalar.activation(o_sb[:], ps[:], ACT.Gelu)
                        nc.sync.dma_start(
                            out[qt * 128:(qt + 1) * 128,
                                dbl * 512:(dbl + 1) * 512],
                            o_sb[:],
                        )
    nc.compile()
    return nc


def _get_nc():
    global _CACHED_NC
    if _CACHED_NC is None:
        _CACHED_NC = _build()
    return _CACHED_NC


def _to_dt(a):
    if COMPUTE_DT == "bf16":
        import ml_dtypes
        return np.ascontiguousarray(a, dtype=ml_dtypes.bfloat16)
    return np.ascontiguousarray(a, dtype=np.float32)


def kernel(value, key_t, query, mask, W_q, W_k, W_v, W_o, b_o):
    from concourse.bass_utils import run_bass_kernel_spmd

    nc = _get_nc()

    value = np.asarray(value, dtype=np.float32)
    key_t = np.asarray(key_t, dtype=np.float32)
    query = np.asarray(query, dtype=np.float32)
    WqT = _to_dt(np.asarray(W_q, np.float32).T)
    WkT = _to_dt(np.asarray(W_k, np.float32).T)
    WvT = _to_dt(np.asarray(W_v, np.float32).T)
    WoT = _to_dt(np.asarray(W_o, np.float32).T)
    bo = _to_dt(np.asarray(b_o, np.float32).reshape(1, D))

    in_maps = []
    for c in range(NCORES):
        b, j = divmod(c, 2)
        qT = _to_dt(query[b].T[:, j * QLEN:(j + 1) * QLEN])
        kT = _to_dt(key_t[b].T[:, j * QLEN:(j + 1) * QLEN])
        vT = _to_dt(value[b].T[:, j * QLEN:(j + 1) * QLEN])
        in_maps.append({
            "qT_in": qT, "kT_in": kT, "vT_in": vT,
            "WqT": WqT, "WkT": WkT, "WvT": WvT, "WoT": WoT, "b_o": bo,
        })

    res = run_bass_kernel_spmd(nc, in_maps, core_ids=list(range(NCORES)))

    out = np.empty((B, S, D), np.float32)
    for c in range(NCORES):
        b, j = divmod(c, 2)
        out[b, j * QLEN:(j + 1) * QLEN, :] = res.results[c]["out"]
    # stash for test harness introspection
    kernel.last_results = res
    return out
